# revision 43
# baseline (speedup 1.0000x reference)
"""Single-head attention (B=4, S=4096, D=1024, K=128) on 8 TRN2 NeuronCores.

Sharding: batch (4) x query-half (2) = 8 shards. Each core computes K/V
projections over the full sequence of its batch element and attention for
its 2048 query rows. No collectives needed.

Per-core layout (everything transposed so no on-chip attn transposes):
  xt   [128, slab, dtile, s]  X^T retiled on host (contiguous 8KB DMA lines),
                              with the core's q-half seq positions first
  KT/QT[128, S]/[128, QH]     k-dim on partitions, bf16
  V    [s, kd] bf16 via DMA-xbar transposes (4x [128,1024] -> 8 tiles each,
                              issued from sync; zero PE transposes)
  ST   [s, q] score tiles = KT_tile.T @ QT  (fp32 PSUM, groups of 3 s-tiles,
                              double-buffered: 6 banks + 2 AV banks = 8)
  est  exp(ST*scale - 12) bf16 (global shift cancels in the softmax ratio)
  OT   [kd, q] = sum_s V_tile.T @ est  -> bf16 out (host transposes back)

Schedule:
  - HAM warm-up: one junk-matmul accumulation group (gap-free 107ns cadence)
    on a memset tile during the input-DMA window + early exp-table preload.
  - All input DMAs on the sync HWDGE ring (stripes across 16 DMA engines;
    the scalar ring does not), ordered so per-slab K/V (+Q0,Q1) consumption
    paces the slab arrivals.
  - Era A: per-slab K/V/Q projections + qtile-0 scores interleaved.
  - Eras B/C/D: skewed AV(q) | scores/exp(q+1); Q2/Q3 projections run from
    free ot-pool PSUM slots early in eras B/C so era boundaries never
    serialize (proj -> copy -> scores -> exp); era D merges AV(2), scores(3)
    (one-group lead) and AV(3); the final exp is split in halves to shorten
    the tail chain.
  - Denominators: DVE accumulates est per qtile (fp16), folds to [128,512];
    the host does the partition sum and the divide (it already transposes).
Engines: PE matmuls only; ACT exp + kt/qt copies; DVE racc/folds/casts;
sync DMAs + xbar transposes. PE is the bottleneck at ~89us of bf16 streaming.
"""
import sys
import types
import numpy as np

B, S, D, KD = 4, 4096, 1024, 128
QH = S // 2              # queries per core
SCALE = 1.0 / np.sqrt(KD)
SHIFT = -12.0            # global exp shift; cancels in softmax ratio
N_SLAB = 8               # seq slabs of 512 for projections
SLAB = S // N_SLAB       # 512
ND = D // 128            # 8 d-tiles
N_ST = S // 128          # 32 s-tiles of 128
QT_TILE = 512            # q tile width
N_QT = QH // QT_TILE     # 4
# score/exp groups: 3 s-tiles per group (3 PSUM banks, x2 buffered), last=2
GRPS = [(3 * i, 3) for i in range(10)] + [(30, 2)]
N_GRP = len(GRPS)        # 11
N_WARM = 52              # HAM warm-up junk matmuls (N=128, ~107ns cold each)

_MAX_WAITS = 1


def _install_shims():
    """Environment fixes: NTFF profiling hook under axon + walrus sync-wait cap."""
    import concourse.bass_utils as bu
    try:
        import antenv.axon_hooks  # noqa: F401
    except ImportError:
        try:
            import trn_agent_boot.trn_boot as tb
            hook = tb._ntff_profile_via_ctypes('/opt/axon/libaxon_pjrt.so')
        except Exception:
            hook = None
        mod = types.ModuleType('antenv.axon_hooks')
        mod.get_axon_ntff_profile_hook = lambda: hook
        mod.set_axon_ntff_profile_hook = lambda h: None
        sys.modules['antenv.axon_hooks'] = mod
        import antenv
        antenv.axon_hooks = mod
    bu.upload_artifacts = lambda tmpdir: tmpdir

    import concourse.tile as tile
    import concourse.mybir as mybir
    from concourse.vector_clock import ScopedClock

    def _drain_and_barrier(self, tick_clock, wait_clock):
        nc = self.nc
        # The walrus build here only accepts 1 sync-wait per CTRL instruction;
        # spread the tail drain's waits over preceding single-wait NOPs.
        _engs = [nc.sync, nc.vector, nc.scalar, nc.tensor, nc.gpsimd]
        nops = [_engs[i % 5].nop(nofuse=True, hint=f"predrain{i}")
                for i in range(11)]
        drain_inst = nc.sync.drain()
        wait_clock.add_sem_waits(
            drain_inst.ins, ScopedClock({None: tick_clock.global_clock})
        )
        waits = list(drain_inst.ins.sync_info.on_wait or [])
        if len(waits) > _MAX_WAITS:
            drain_inst.ins.sync_info.on_wait = waits[:_MAX_WAITS - 1] if _MAX_WAITS > 1 else []
            rest = waits[_MAX_WAITS - 1:] if _MAX_WAITS > 1 else waits
            for i, nop in enumerate(nops):
                chunk = rest[i * _MAX_WAITS:(i + 1) * _MAX_WAITS]
                if chunk:
                    if nop.ins.sync_info is None:
                        nop.ins.sync_info = mybir.SyncInfo(on_wait=chunk, on_update=[])
                    else:
                        nop.ins.sync_info.on_wait = chunk
        nc.all_engine_barrier()
        assert self.sems is not None
        popped = nc._tile_sem_poison_stack.pop()
        assert popped is self._sem_poison
        nc.clear_and_free_semaphores(list(self.sems.allocated().values()))
        nc.all_engine_barrier()

    tile.TileContext._drain_and_barrier = _drain_and_barrier


def _split_waits(nc):
    """This walrus build accepts at most 1 sync-wait per instruction; hoist
    excess waits onto same-engine NoOps inserted immediately before."""
    import concourse.mybir as mybir
    ctr = [0]
    for fn in nc.m.functions:
        for blk in fn.blocks:
            insts = blk.instructions
            out = []
            for inst in insts:
                si = getattr(inst, "sync_info", None)
                waits = list(si.on_wait) if si is not None and si.on_wait else []
                if len(waits) > 1:
                    for w in waits[1:]:
                        ctr[0] += 1
                        nop = mybir.InstNoOp(name=f"I-ws{ctr[0]}", ins=[], outs=[])
                        nop.engine = inst.engine
                        nop.sync_info = mybir.SyncInfo(on_wait=[w], on_update=[])
                        out.append(nop)
                    si.on_wait = waits[:1]
                out.append(inst)
            if len(out) != len(insts):
                insts.clear()
                insts.extend(out)
    return nc


def build_graph():
    import concourse.bass as bass
    import concourse.mybir as mybir
    import concourse.tile as tile
    dt = mybir.dt
    f32, bf16, f16 = dt.float32, dt.bfloat16, dt.float16
    EXP = mybir.ActivationFunctionType.Exp

    nc = bass.Bass()
    xt = nc.declare_dram_parameter("xt", [128, N_SLAB, ND, SLAB], bf16, isOutput=False).ap()
    wq = nc.declare_dram_parameter("wq", [128, ND, KD], bf16, isOutput=False).ap()
    wk = nc.declare_dram_parameter("wk", [128, ND, KD], bf16, isOutput=False).ap()
    wv = nc.declare_dram_parameter("wv", [128, ND, KD], bf16, isOutput=False).ap()
    out = nc.declare_dram_parameter("out", [N_QT, KD, QT_TILE], bf16, isOutput=True).ap()
    rout = nc.declare_dram_parameter("rout", [N_QT, 128, QT_TILE], f16, isOutput=True).ap()

    with tile.TileContext(nc) as tc:
        with (
            tc.tile_pool(name="wm", bufs=1) as wmp,
            tc.tile_pool(name="w", bufs=3) as wp,
            tc.tile_pool(name="kt", bufs=1) as ktp,
            tc.tile_pool(name="qt", bufs=1) as qtp,
            tc.tile_pool(name="v", bufs=1) as vp,
            tc.tile_pool(name="xts", bufs=N_SLAB - 1) as xtp,
            tc.tile_pool(name="x0", bufs=4) as x0p,
            tc.tile_pool(name="st", bufs=2, space="PSUM") as stp,
            tc.tile_pool(name="est", bufs=16) as estp,
            tc.tile_pool(name="racc", bufs=2) as raccp,
            tc.tile_pool(name="vts", bufs=4) as vtsp,
        ):
            # ---- phase 0: warm-up tiles + exp table preload ----
            warm = wmp.tile([128, 128], bf16, tag="warm")
            nc.vector.memset(warm[:], 0.0)
            scrap = wmp.tile([128, 1], bf16, tag="scrap")
            bias_sb = wmp.tile([128, 1], f32, tag="bias")
            nc.vector.memset(bias_sb[:], float(SHIFT))
            # first ACTIVATE carries the exp table load (~2.7us): do it now,
            # during the DMA window, not on the first score group
            nc.scalar.activation(
                scrap[:], warm[:, 0:1], EXP, scale=1.0, bias=bias_sb[:])

            # ---- phase 1: input DMAs, all on the sync HWDGE ring (it
            # stripes across all 16 DMA engines; scalar's ring does not) ----
            w_sb = {}
            for n in ("wq", "wk", "wv"):
                w_sb[n] = wp.tile([128, D], bf16, tag="w", name=f"w_{n}")
            x0ps = [
                x0p.tile([128, 2 * SLAB], bf16, tag="x0", name=f"x0{h}")
                for h in range(4)
            ]

            def dma_x0(h):
                nc.sync.dma_start(
                    x0ps[h][:].rearrange("p (t s) -> p t s", t=2),
                    xt[:, 0, 2 * h:2 * h + 2])

            def dma_w(n, w_ap):
                nc.sync.dma_start(
                    w_sb[n][:].rearrange("p (t k) -> p t k", t=ND), w_ap)

            dma_x0(0)
            dma_x0(1)
            dma_w("wk", wk)
            dma_x0(2)
            dma_x0(3)
            dma_w("wq", wq)
            dma_w("wv", wv)
            xts = [None]
            for j in range(1, N_SLAB):
                x_t = xtp.tile([128, ND * SLAB], bf16, tag="xts", name=f"xts{j}")
                nc.sync.dma_start(
                    x_t[:].rearrange("p (t s) -> p t s", t=ND), xt[:, j])
                xts.append(x_t)

            kt_sb = ktp.tile([128, S], bf16)
            qt_sb = qtp.tile([128, QH], bf16)
            v_sb = vp.tile([128, S], bf16)

            est_tiles = [[None] * N_GRP for _ in range(N_QT)]
            racc_t = [None] * N_QT

            def xsrc(j, d):
                if j == 0:
                    return x0ps[d // 2][:, (d % 2) * SLAB:(d % 2 + 1) * SLAB]
                return xts[j][:, d * SLAB:(d + 1) * SLAB]

            def score_exp_group(q, g):
                s0, gsz = GRPS[g]
                w_ = gsz * QT_TILE
                stps = stp.tile([128, 3 * QT_TILE], f32, tag="st",
                                name=f"st_{q}_{g}")
                qs = slice(q * QT_TILE, (q + 1) * QT_TILE)
                for i in range(gsz):
                    st_i = s0 + i
                    nc.tensor.matmul(
                        stps[:, i * QT_TILE:(i + 1) * QT_TILE],
                        kt_sb[:, st_i * 128:(st_i + 1) * 128],
                        qt_sb[:, qs],
                        start=True, stop=True,
                    )
                est = estp.tile([128, 3 * QT_TILE], bf16, tag="est",
                                name=f"est_{q}_{g}")
                if q == 3 and g == N_GRP - 1:
                    # tail path: two half-exps so the final AV/fold chain
                    # starts one half earlier
                    nc.scalar.activation(
                        est[:, :QT_TILE], stps[:, :QT_TILE], EXP,
                        scale=float(SCALE), bias=bias_sb[:])
                    nc.scalar.activation(
                        est[:, QT_TILE:w_], stps[:, QT_TILE:w_], EXP,
                        scale=float(SCALE), bias=bias_sb[:])
                else:
                    nc.scalar.activation(
                        est[:, :w_], stps[:, :w_], EXP, scale=float(SCALE),
                        bias=bias_sb[:])
                est_tiles[q][g] = est
                if g == 1:
                    racc = raccp.tile([128, 3 * QT_TILE], f16, tag="racc",
                                      name=f"racc{q}")
                    racc_t[q] = racc
                    nc.vector.tensor_add(
                        racc[:], est_tiles[q][0][:], est[:])
                elif g > 1 and not (q == 3 and g == N_GRP - 1):
                    # qtile 3's last group folds straight into rf (tail path)
                    nc.vector.tensor_add(
                        racc_t[q][:, :w_], racc_t[q][:, :w_], est[:, :w_])

            with tc.tile_pool(name="pp", bufs=2, space="PSUM") as pp:
                # ---- phase 0b: HAM warm-up junk matmuls during DMA wait;
                # one accumulation group = no inter-MM deps, pure 107ns cadence
                jt = pp.tile([128, SLAB], f32, tag="pp", name="junk")
                for i in range(N_WARM):
                    nc.tensor.matmul(
                        jt[:, 0:128], warm[:], warm[:],
                        start=(i == 0), stop=(i == N_WARM - 1))

                def proj(name, j, dst, dslice):
                    ps = pp.tile([128, SLAB], f32, tag="pp", name=f"ps_{name}{j}")
                    for d in range(ND):
                        nc.tensor.matmul(
                            ps[:],
                            w_sb[name][:, d * 128:(d + 1) * 128],
                            xsrc(j, d),
                            start=(d == 0), stop=(d == ND - 1),
                        )
                    nc.scalar.copy(dst[:, dslice], ps[:])

                vt_pair = [None] * 4

                def v_slab(j):
                    ps = pp.tile([128, SLAB], f32, tag="pp", name=f"psv{j}")
                    for d in range(ND):
                        nc.tensor.matmul(
                            ps[:],
                            w_sb["wv"][:, d * 128:(d + 1) * 128],
                            xsrc(j, d),
                            start=(d == 0), stop=(d == ND - 1),
                        )
                    if j % 2 == 0:
                        vt_pair[j // 2] = vtsp.tile(
                            [128, 2 * SLAB], bf16, tag="vts", name=f"vt{j // 2}")
                    nc.vector.tensor_copy(
                        vt_pair[j // 2][:, (j % 2) * SLAB:(j % 2 + 1) * SLAB],
                        ps[:])

                def v_transpose(jp):
                    # one xbar transpose per slab-pair: V^T [kd, 1024 s] ->
                    # 8 V tiles [s, kd]  (out[p,t,c] = in[c, t*128+p])
                    nc.sync.dma_start_transpose(
                        v_sb[:, jp * 8 * 128:(jp + 1) * 8 * 128].rearrange(
                            "p (t c) -> p t c", t=8),
                        vt_pair[jp][:])

                def av_group(q, g, ot):
                    s0, gsz = GRPS[g]
                    est = est_tiles[q][g]
                    for i in range(gsz):
                        st_i = s0 + i
                        nc.tensor.matmul(
                            ot[:],
                            v_sb[:, st_i * 128:(st_i + 1) * 128],
                            est[:, i * QT_TILE:(i + 1) * QT_TILE],
                            start=(st_i == 0), stop=(st_i == N_ST - 1),
                        )

                # ---- era A: per-slab K/V (+Q0,Q1) so PE consumption
                # paces the slab DMA arrivals; qtile-0 scores interleaved.
                # Q2/Q3 are deferred into the ACT-bound eras below.
                proj("wk", 0, kt_sb, slice(0, SLAB))
                proj("wq", 0, qt_sb, slice(0, SLAB))
                v_slab(0)
                score_exp_group(0, 0)
                proj("wk", 1, kt_sb, slice(SLAB, 2 * SLAB))
                proj("wq", 1, qt_sb, slice(SLAB, 2 * SLAB))
                v_slab(1)
                v_transpose(0)
                score_exp_group(0, 1)
                proj("wk", 2, kt_sb, slice(2 * SLAB, 3 * SLAB))
                v_slab(2)
                score_exp_group(0, 2)
                score_exp_group(0, 3)
                proj("wk", 3, kt_sb, slice(3 * SLAB, 4 * SLAB))
                v_slab(3)
                v_transpose(1)
                score_exp_group(0, 4)
                proj("wk", 4, kt_sb, slice(4 * SLAB, 5 * SLAB))
                v_slab(4)
                score_exp_group(0, 5)
                proj("wk", 5, kt_sb, slice(5 * SLAB, 6 * SLAB))
                v_slab(5)
                v_transpose(2)
                score_exp_group(0, 6)
                score_exp_group(0, 7)
                proj("wk", 6, kt_sb, slice(6 * SLAB, 7 * SLAB))
                v_slab(6)
                score_exp_group(0, 8)
                proj("wk", 7, kt_sb, slice(7 * SLAB, 8 * SLAB))
                v_slab(7)
                v_transpose(3)
                score_exp_group(0, 9)
                score_exp_group(0, 10)

            # ---- eras B/C/D: skewed AV(q) | scores/exp(q+1) ----
            with (
                tc.tile_pool(name="ot", bufs=2, space="PSUM") as otp,
                tc.tile_pool(name="rf", bufs=2) as rfp,
                tc.tile_pool(name="osb", bufs=2) as osbp,
            ):
                def av_group(q, g, ot):
                    s0, gsz = GRPS[g]
                    est = est_tiles[q][g]
                    for i in range(gsz):
                        st_i = s0 + i
                        nc.tensor.matmul(
                            ot[:],
                            v_sb[:, st_i * 128:(st_i + 1) * 128],
                            est[:, i * QT_TILE:(i + 1) * QT_TILE],
                            start=(st_i == 0), stop=(st_i == N_ST - 1),
                        )

                def denom_out(q):
                    rf = rfp.tile([128, QT_TILE], f16, tag="rf", name=f"rf{q}")
                    nc.vector.tensor_add(
                        rf[:], racc_t[q][:, 0:QT_TILE],
                        racc_t[q][:, QT_TILE:2 * QT_TILE])
                    nc.vector.tensor_add(
                        rf[:], rf[:], racc_t[q][:, 2 * QT_TILE:3 * QT_TILE])
                    if q == 3:
                        est10 = est_tiles[3][N_GRP - 1]
                        nc.vector.tensor_add(
                            rf[:], rf[:], est10[:, 0:QT_TILE])
                        nc.vector.tensor_add(
                            rf[:], rf[:], est10[:, QT_TILE:2 * QT_TILE])
                    nc.sync.dma_start(rout[q], rf[:])

                def out_dma(q, ot):
                    o_sb = osbp.tile([128, QT_TILE], bf16, tag="osb",
                                     name=f"os{q}")
                    if q == 3:
                        # after the last exp the ACT engine is idle; DVE is
                        # still busy with the denominator folds
                        nc.scalar.copy(o_sb[:], ot[:])
                    else:
                        nc.vector.tensor_copy(o_sb[:], ot[:])
                    nc.sync.dma_start(out[q], o_sb[:])

                def proj_ot(name, j, dst, dslice):
                    # deferred projection using a free ot-pool slot
                    ps = otp.tile([128, QT_TILE], f32, tag="ot",
                                  name=f"po_{name}{j}")
                    for d in range(ND):
                        nc.tensor.matmul(
                            ps[:],
                            w_sb[name][:, d * 128:(d + 1) * 128],
                            xts[j][:, d * SLAB:(d + 1) * SLAB],
                            start=(d == 0), stop=(d == ND - 1),
                        )
                    nc.scalar.copy(dst[:, dslice], ps[:])

                # era B: AV(0) | scores(1); denom(0) ready from era A
                ot0 = otp.tile([128, QT_TILE], f32, tag="ot", name="ot0")
                denom_out(0)
                for g in range(N_GRP):
                    av_group(0, g, ot0)
                    score_exp_group(1, g)
                    if g == 1:
                        # Q2 early (ot slot s1 is free all era): its copy
                        # lands mid-era so sc(2,0) can fire right at the
                        # boundary instead of serializing into it
                        proj_ot("wq", 2, qt_sb, slice(2 * SLAB, 3 * SLAB))
                out_dma(0, ot0)

                # era C: AV(1) | scores(2)
                ot1 = otp.tile([128, QT_TILE], f32, tag="ot", name="ot1")
                denom_out(1)
                for g in range(N_GRP):
                    av_group(1, g, ot1)
                    score_exp_group(2, g)
                    if g == 1:
                        proj_ot("wq", 3, qt_sb, slice(3 * SLAB, 4 * SLAB))
                out_dma(1, ot1)

                # era D: AV(2) | scores(3) | AV(3) merged
                ot2 = otp.tile([128, QT_TILE], f32, tag="ot", name="ot2")
                ot3 = otp.tile([128, QT_TILE], f32, tag="ot", name="ot3")
                denom_out(2)
                score_exp_group(3, 0)
                for g in range(N_GRP):
                    av_group(2, g, ot2)
                    if g + 1 < N_GRP:
                        score_exp_group(3, g + 1)
                    else:
                        out_dma(2, ot2)
                        denom_out(3)
                    av_group(3, g, ot3)
                out_dma(3, ot3)
    _split_waits(nc)
    return nc


_CACHED = {}


def kernel(input_vec, weight_query, weight_key, weight_value):
    _install_shims()
    from concourse.bass_utils import run_bass_kernel_spmd
    import ml_dtypes

    bf16 = ml_dtypes.bfloat16
    x = np.asarray(input_vec, dtype=np.float32)
    wq = np.asarray(weight_query, dtype=np.float32)
    wk = np.asarray(weight_key, dtype=np.float32)
    wv = np.asarray(weight_value, dtype=np.float32)

    def wtile(w):
        # [D, KD] -> [p, t, k] with d = t*128 + p
        return np.ascontiguousarray(
            w.reshape(ND, 128, KD).transpose(1, 0, 2).astype(bf16))

    wq_t, wk_t, wv_t = wtile(wq), wtile(wk), wtile(wv)

    if "nc" not in _CACHED:
        _CACHED["nc"] = build_graph()
    nc = _CACHED["nc"]

    in_maps = []
    for c in range(8):
        b, h = c // 2, c % 2
        qlo, qhi = h * QH, (h + 1) * QH
        # seq reorder: this core's q rows first (softmax is order-invariant)
        xs = np.concatenate([x[b, qlo:qhi], x[b, :qlo], x[b, qhi:]], axis=0)
        # [S, D] -> [p, slab, t, s] with seq = slab*512 + s, d = t*128 + p
        xt_c = np.ascontiguousarray(
            xs.reshape(N_SLAB, SLAB, ND, 128).transpose(3, 0, 2, 1).astype(bf16))
        in_maps.append({"xt": xt_c, "wq": wq_t, "wk": wk_t, "wv": wv_t})

    import os
    trace = bool(os.environ.get("KERNEL_TRACE"))
    res = run_bass_kernel_spmd(nc, in_maps, list(range(8)), trace=trace)
    _CACHED["last_exec_time_ns"] = res.exec_time_ns
    _CACHED["last_results"] = res

    out = np.empty((B, S, KD), dtype=np.float32)
    for c in range(8):
        b, h = c // 2, c % 2
        # denominators: partition-sum the folded est accumulators
        r = res.results[c]["rout"].astype(np.float32).sum(axis=1).reshape(QH)
        o = res.results[c]["out"].astype(np.float32)
        o = o.transpose(1, 0, 2).reshape(KD, QH)
        out[b, h * QH:(h + 1) * QH, :] = o.T / r[:, None]
    return out
